# revision 22
# baseline (speedup 1.0000x reference)
"""GPT-Neo self-attention on 8 NeuronCores (Trainium2, Bass/Tile) — v11.

Sharding: core i handles batch i//4 and head-group i%4 (3 of 12 heads).
Each core computes a partial out-projection [S, D] (bf16); host sums the
4 partials per batch in f32.

v11 vs v10 (all driven by the HW trace):
- Head: inputs stream as per-k-tile DMA slices on FOUR queues
  (sync/vector/gpsimd/scalar); proj(0)/proj(1) accumulate k-major so the
  first matmul fires as soon as slice k=0 lands (~6us vs 13.2us).
  Warmup matmuls during the DMA wait keep the PE DVFS clock hot.
- PE p-state: every semaphore wait on the PE resets its DVFS ramp
  (observed quantized 216/318/427ns for N=512).  So: LAG 4->6, the two
  Pool tri-mask multiplies per diag pair merged into ONE strided-AP
  multiply, diag-B score sections packed contiguously (1 exp not 2),
  posb/copies moved off the hot engines, `den` staging copy dropped
  (reciprocal reads PSUM row directly).
- Phase boundaries: the last tail's normalize chain (~2us serial) is
  deferred into the next att() phase; att(3)'s drain is filled with
  outproj work.
- Endgame: chunk-3 onp/ons live as per-128-column tiles so op12..15
  start as soon as the first column piece is normalized.
"""

import numpy as np
import ml_dtypes
from collections import deque
from contextlib import ExitStack

import concourse.bass as bass
from concourse import bacc
import concourse.mybir as mybir
import concourse.tile as tile
from concourse.bass_utils import run_bass_kernel_spmd

B, S, D, H = 2, 2048, 768, 12
HD = 64
HPC = 3
NCORES = 8
NEG = -1.0e30
F32 = mybir.dt.float32
BF16 = mybir.dt.bfloat16
EXP = mybir.ActivationFunctionType.Exp
COPY = mybir.ActivationFunctionType.Copy

KT = D // 128
SQT = S // 128
CH = S // 512
LAG = 6          # main-PV pair-units behind scores
WARM = 8         # PE warmup matmuls during the head DMA wait


def build_nc(use_pbias=False):
    nc = bacc.Bacc(None, target_bir_lowering=False)

    xT = [nc.declare_dram_parameter(f"xT{c}", [128, KT, 512], BF16,
                                    isOutput=False) for c in range(CH)]
    wqk = nc.declare_dram_parameter("wqk", [128, KT, 384], BF16, isOutput=False)
    wv = nc.declare_dram_parameter("wv", [128, KT, 192], BF16, isOutput=False)
    wop = nc.declare_dram_parameter("wop", [128, D], BF16, isOutput=False)
    wos = nc.declare_dram_parameter("wos", [65, D], BF16, isOutput=False)
    trid = nc.declare_dram_parameter("trid", [128, 2, 128], BF16, isOutput=False)
    if use_pbias:
        pbias = nc.declare_dram_parameter("pbias", [128, SQT], F32, isOutput=False)
    y = nc.declare_dram_parameter("y", [S, D], BF16, isOutput=True)

    with tile.TileContext(nc) as tc:
        with ExitStack() as ctx:
            persist = ctx.enter_context(tc.tile_pool(name="persist", bufs=1))
            ptp = ctx.enter_context(tc.tile_pool(name="ptp", bufs=14))
            recp = ctx.enter_context(tc.tile_pool(name="recp", bufs=3))
            posp = ctx.enter_context(tc.tile_pool(name="posp", bufs=2))
            big = ctx.enter_context(tc.tile_pool(name="big", bufs=3, space="PSUM"))
            pop = ctx.enter_context(tc.tile_pool(name="pop", bufs=2, space="PSUM"))

            # per-(chunk,k) x tiles so k-major proj never waits a later slice
            xck = [[persist.tile([128, 512], BF16, tag=f"x{c}_{k}",
                                 name=f"x{c}_{k}") for k in range(KT)]
                   for c in range(2)]
            xc = [None, None] + [persist.tile([128, KT, 512], BF16, tag=f"xc{c}",
                                              name=f"xc{c}") for c in (2, 3)]
            wqk_k = [persist.tile([128, 384], BF16, tag=f"wqk{k}",
                                  name=f"wqk{k}") for k in range(KT)]
            wv_sb = persist.tile([128, KT, 192], BF16, tag="wv", name="wv")
            wop_sb = persist.tile([128, D], BF16, tag="wop", name="wop")
            wos_sb = persist.tile([65, D], BF16, tag="wos", name="wos")
            tri_sb = persist.tile([128, 2, 128], BF16, tag="tri", name="tri")
            ones65 = persist.tile([1, 65], BF16, tag="ones65", name="ones65")
            wsrc = persist.tile([128, 256], BF16, tag="wsrc", name="wsrc")
            if use_pbias:
                pb_sb = persist.tile([128, SQT], F32, tag="pb", name="pb")
            # per-chunk activation tiles (avoid false tile-level deps)
            q01c = [persist.tile([128, 512], BF16, tag=f"q01_{c}", name=f"q01_{c}")
                    for c in range(CH)]
            k01c = [persist.tile([128, 512], BF16, tag=f"k01_{c}", name=f"k01_{c}")
                    for c in range(CH)]
            q2c = [persist.tile([64, 512], BF16, tag=f"q2_{c}", name=f"q2_{c}")
                   for c in range(CH)]
            k2c = [persist.tile([64, 512], BF16, tag=f"k2_{c}", name=f"k2_{c}")
                   for c in range(CH)]
            vc = [persist.tile([128, HPC, 4, 65], BF16, tag=f"v{c}", name=f"v{c}")
                  for c in range(CH)]
            onp = [persist.tile([128, 512], BF16, tag=f"onp{c}", name=f"onp{c}")
                   for c in range(CH - 1)]
            ons = [persist.tile([65, 512], BF16, tag=f"ons{c}", name=f"ons{c}")
                   for c in range(CH - 1)]
            # chunk 3: per-128-col pieces so op12..15 start per piece
            onp3t = [persist.tile([128, 128], BF16, tag=f"onp3_{t}",
                                  name=f"onp3_{t}") for t in range(4)]
            ons3t = [persist.tile([65, 128], BF16, tag=f"ons3_{t}",
                                  name=f"ons3_{t}") for t in range(4)]
            otc = [persist.tile([128, 4, D], BF16, tag=f"ot{c}", name=f"ot{c}")
                   for c in range(CH)]

            # ---- head DMA plan ----
            # sync + scalar are HW-DGE (low latency) -> the critical per-k
            # slices; gpsimd is SW-DGE (~7us latency) -> latency-tolerant
            # bulk only.  DVE queue leads with the warmup-source memset.
            nc.vector.memset(wsrc[:], 0.25)
            nc.vector.memset(vc[0][:], 1.0)
            for k in range(KT):                       # proj(0) inputs
                (nc.sync if k % 2 == 0 else nc.scalar).dma_start(
                    out=wqk_k[k][:], in_=wqk[:, k, :])
                (nc.scalar if k % 2 == 0 else nc.sync).dma_start(
                    out=xck[0][k][:], in_=xT[0][:, k, :])
            for k in range(KT):                       # proj(1) inputs
                (nc.sync if k % 2 == 0 else nc.scalar).dma_start(
                    out=xck[1][k][:], in_=xT[1][:, k, :])
            nc.gpsimd.dma_start(out=wv_sb[:], in_=wv[:, :, :])
            nc.gpsimd.dma_start(out=tri_sb[:], in_=trid[:, :, :])
            # chunk 2/3 x + weights, balanced across queues
            nc.gpsimd.dma_start(out=xc[2][:, 0:3, :], in_=xT[2][:, 0:3, :])
            nc.sync.dma_start(out=xc[2][:, 3:6, :], in_=xT[2][:, 3:6, :])
            nc.gpsimd.dma_start(out=xc[3][:, 0:3, :], in_=xT[3][:, 0:3, :])
            nc.scalar.dma_start(out=xc[3][:, 3:6, :], in_=xT[3][:, 3:6, :])
            nc.gpsimd.dma_start(out=wop_sb[:], in_=wop[:, :])
            nc.gpsimd.dma_start(out=wos_sb[:], in_=wos[:, :])
            if use_pbias:
                nc.scalar.dma_start(out=pb_sb[:], in_=pbias[:, :])
            for c in range(1, CH):
                nc.vector.memset(vc[c][:], 1.0)
            nc.vector.memset(ones65[:], 1.0)

            _warm_n = [0]

            def warm(n):
                """Junk matmuls that keep the PE DVFS ramp hot during DMA
                waits; WAW on the rotating pop bufs is in-order on PE."""
                for _ in range(n):
                    w = _warm_n[0]
                    _warm_n[0] += 1
                    wps = pop.tile([65, 512], F32, tag="po", name=f"warm{w}")
                    nc.tensor.matmul(out=wps[:, 0:256], lhsT=wsrc[:, 0:65],
                                     rhs=wsrc[:, :], start=True, stop=True)

            def proj_kmajor(c):
                """k-major qkv projection for chunk c, DMA-paced: warmup
                matmuls between k groups keep the clock up during waits."""
                ps = [big.tile([128, 1024], F32, tag="big", name=f"p{off}_{c}")
                      for off in (0, 128, 256)]
                for k in range(KT):
                    for gi, off in enumerate((0, 128, 256)):
                        nc.tensor.matmul(
                            out=ps[gi][:, 0:512],
                            lhsT=wqk_k[k][:, off:off + 128],
                            rhs=xck[c][k][:],
                            start=(k == 0), stop=(k == KT - 1))
                    if k < KT - 1:
                        warm(2)
                # q01 gates the pv psum-buf reuse: DVE's queue is free
                # earliest at the head (gpsimd cannot read PSUM)
                nc.vector.tensor_copy(out=q01c[c][:], in_=ps[0][:, 0:512])
                nc.scalar.copy(out=k01c[c][:], in_=ps[1][:, 0:512])
                nc.scalar.copy(out=q2c[c][:], in_=ps[2][0:64, 0:512])
                nc.vector.tensor_copy(out=k2c[c][:], in_=ps[2][64:128, 0:512])
                warm(2)  # cover the wv-DMA / q01-copy wait before v groups
                for jj in range(4):
                    pv = big.tile([128, 1024], F32, tag="big", name=f"pv{c}_{jj}")
                    for k in range(KT):
                        nc.tensor.matmul(
                            out=pv[:, 0:192],
                            lhsT=xck[c][k][:, 128 * jj:128 * (jj + 1)],
                            rhs=wv_sb[:, k, :],
                            start=(k == 0), stop=(k == KT - 1))
                    for h in range(HPC):
                        nc.vector.tensor_copy(out=vc[c][:, h, jj, 0:64],
                                              in_=pv[:, 64 * h:64 * (h + 1)])

            def xap(c, k, lo=0, hi=512):
                return (xck[c][k][:, lo:hi] if c < 2
                        else xc[c][:, k, lo:hi])

            def proj_qk_group(c, off):
                ps = big.tile([128, 1024], F32, tag="big", name=f"p{off}_{c}")
                for k in range(KT):
                    nc.tensor.matmul(
                        out=ps[:, 0:512],
                        lhsT=wqk_k[k][:, off:off + 128],
                        rhs=xap(c, k),
                        start=(k == 0), stop=(k == KT - 1))
                if off == 0:
                    nc.vector.tensor_copy(out=q01c[c][:], in_=ps[:, 0:512])
                elif off == 128:
                    nc.vector.tensor_copy(out=k01c[c][:], in_=ps[:, 0:512])
                else:
                    nc.vector.tensor_copy(out=q2c[c][:], in_=ps[0:64, 0:512])
                    nc.vector.tensor_copy(out=k2c[c][:], in_=ps[64:128, 0:512])

            def proj_v_group(c, jj):
                pv = big.tile([128, 1024], F32, tag="big", name=f"pv{c}_{jj}")
                for k in range(KT):
                    nc.tensor.matmul(
                        out=pv[:, 0:192],
                        lhsT=xap(c, k, 128 * jj, 128 * (jj + 1)),
                        rhs=wv_sb[:, k, :],
                        start=(k == 0), stop=(k == KT - 1))
                for h in range(HPC):
                    nc.vector.tensor_copy(out=vc[c][:, h, jj, 0:64],
                                          in_=pv[:, 64 * h:64 * (h + 1)])

            def proj_groups(c):
                gs = [lambda off=off: proj_qk_group(c, off)
                      for off in (0, 128, 256)]
                gs += [lambda jj=jj: proj_v_group(c, jj) for jj in range(4)]
                return gs

            def kq(h, j):
                """(k-block lhsT) for head h, key tile j."""
                cj, jj = j // 4, j % 4
                if h == 2:
                    return k2c[cj][:, 128 * jj:128 * (jj + 1)]
                lo = 64 * h
                return k01c[cj][lo:lo + 64, 128 * jj:128 * (jj + 1)]

            def qv(h, c, lo, hi):
                if h == 2:
                    return q2c[c][:, lo:hi]
                p0 = 64 * h
                return q01c[c][p0:p0 + 64, lo:hi]

            def v_ap(h, j):
                return vc[j // 4][:, h, j % 4, :]

            def exp_emit(pt, Sg, sections):
                if use_pbias:
                    for lo, hi, j in sections:
                        nc.scalar.activation(out=pt[:, lo:hi], in_=Sg[:, lo:hi],
                                             func=EXP, bias=pb_sb[:, j:j + 1])
                else:
                    lo, hi = sections[0][0], sections[-1][1]
                    nc.scalar.activation(out=pt[:, lo:hi], in_=Sg[:, lo:hi],
                                         func=EXP)

            def tri_mask(pt, stride):
                """one Pool multiply masking cols {0:128, stride:stride+128}"""
                v = pt[:].rearrange("p (a b) -> p a b", b=stride)[:, 0:2, 0:128]
                nc.gpsimd.tensor_mul(out=v, in0=v, in1=tri_sb[:])

            def outproj(t, tail=False):
                c_, tt = t // 4, t % 4
                Sg = big.tile([128, 1024], F32, tag="big", name=f"op{t}")
                if c_ == 3:
                    lp, ls = onp3t[tt][:], ons3t[tt][:]
                else:
                    ts_ = slice(128 * tt, 128 * (tt + 1))
                    lp, ls = onp[c_][:, ts_], ons[c_][:, ts_]
                for lo in (0, 512):
                    hs = slice(384 * (lo // 512), 384 * (lo // 512) + 384)
                    nc.tensor.matmul(out=Sg[:, lo:lo + 384],
                                     lhsT=lp, rhs=wop_sb[:, hs],
                                     start=True, stop=False)
                    nc.tensor.matmul(out=Sg[:, lo:lo + 384],
                                     lhsT=ls, rhs=wos_sb[:, hs],
                                     start=False, stop=True)
                if tail:
                    # ACT is exp-free here: split the copies across engines
                    # and flush each q-tile as soon as it is staged
                    nc.scalar.activation(out=otc[c_][:, tt, 0:384],
                                         in_=Sg[:, 0:384], func=COPY)
                    nc.vector.tensor_copy(out=otc[c_][:, tt, 384:768],
                                          in_=Sg[:, 512:896])
                    nc.sync.dma_start(
                        out=y[128 * t:128 * (t + 1), :].rearrange(
                            "(t p) d -> p t d", p=128),
                        in_=otc[c_][:, tt:tt + 1, :])
                    return
                nc.vector.tensor_copy(out=otc[c_][:, tt, 0:384], in_=Sg[:, 0:384])
                nc.vector.tensor_copy(out=otc[c_][:, tt, 384:768],
                                      in_=Sg[:, 512:896])
                if tt % 2 == 1:  # flush 2 q-tiles
                    nc.sync.dma_start(
                        out=y[128 * (t - 1):128 * (t + 1), :].rearrange(
                            "(t p) d -> p t d", p=128),
                        in_=otc[c_][:, tt - 1:tt + 1, :])

            def att(c, fillers, pre=None, drain_fillers=(), last_pieces=False):
                npairs = 2 * c + 2
                pts = {}
                po_t = {}

                def emit_S(h, p):
                    Sg = big.tile([128, 1024], F32, tag="big", name=f"S{c}{h}{p}")
                    pt = ptp.tile([128, 1024], BF16, tag="pt", name=f"pt{c}{h}{p}")
                    if p < 2 * c:          # full pair: j = 2p, 2p+1
                        j0 = 2 * p
                        nc.tensor.matmul(
                            out=Sg[:, 0:512], lhsT=kq(h, j0),
                            rhs=qv(h, c, 0, 512), start=True, stop=True)
                        nc.tensor.matmul(
                            out=Sg[:, 512:1024], lhsT=kq(h, j0 + 1),
                            rhs=qv(h, c, 0, 512), start=True, stop=True)
                        exp_emit(pt, Sg, [(0, 512, j0), (512, 1024, j0 + 1)])
                    elif p == 2 * c:       # diag A: j=4c (512 cols), 4c+1 (384)
                        j0 = 4 * c
                        nc.tensor.matmul(
                            out=Sg[:, 0:512], lhsT=kq(h, j0),
                            rhs=qv(h, c, 0, 512), start=True, stop=True)
                        nc.tensor.matmul(
                            out=Sg[:, 512:896], lhsT=kq(h, j0 + 1),
                            rhs=qv(h, c, 128, 512), start=True, stop=True)
                        exp_emit(pt, Sg, [(0, 512, j0), (512, 896, j0 + 1)])
                        tri_mask(pt, 512)
                    else:                  # diag B: j=4c+2 (256 cols), 4c+3 (128)
                        j0 = 4 * c + 2
                        nc.tensor.matmul(
                            out=Sg[:, 0:256], lhsT=kq(h, j0),
                            rhs=qv(h, c, 256, 512), start=True, stop=True)
                        nc.tensor.matmul(
                            out=Sg[:, 256:384], lhsT=kq(h, j0 + 1),
                            rhs=qv(h, c, 384, 512), start=True, stop=True)
                        exp_emit(pt, Sg, [(0, 256, j0), (256, 384, j0 + 1)])
                        tri_mask(pt, 256)
                    pts[(h, p)] = pt

                def emit_P_main(h, p):
                    if c == 0:
                        return  # all PVs deferred (need the masked pt anyway)
                    pt = pts[(h, p)]
                    if p == 0:
                        po_t[h] = pop.tile([65, 512], F32, tag="po",
                                           name=f"po{c}_{h}")
                    po = po_t[h]
                    if p < 2 * c:
                        nc.tensor.matmul(
                            out=po[:, :], lhsT=v_ap(h, 2 * p),
                            rhs=pt[:, 0:512], start=(p == 0), stop=False)
                        nc.tensor.matmul(
                            out=po[:, :], lhsT=v_ap(h, 2 * p + 1),
                            rhs=pt[:, 512:1024], start=False, stop=False)
                        pts.pop((h, p))
                    elif p == 2 * c:
                        nc.tensor.matmul(
                            out=po[:, 128:512], lhsT=v_ap(h, 4 * c),
                            rhs=pt[:, 128:512], start=False, stop=False)
                        nc.tensor.matmul(
                            out=po[:, 256:512], lhsT=v_ap(h, 4 * c + 1),
                            rhs=pt[:, 640:896], start=False, stop=False)
                    else:
                        nc.tensor.matmul(
                            out=po[:, 384:512], lhsT=v_ap(h, 4 * c + 2),
                            rhs=pt[:, 128:256], start=False, stop=False)

                def tail_pv(h):
                    """Deferred triangle PVs (+ all PVs at c==0)."""
                    ptA = pts.pop((h, 2 * c))
                    ptB = pts.pop((h, 2 * c + 1))
                    if c == 0:
                        po_t[h] = pop.tile([65, 512], F32, tag="po",
                                           name=f"po{c}_{h}")
                        po = po_t[h]
                        nc.tensor.matmul(
                            out=po[:, 0:512], lhsT=v_ap(h, 0),
                            rhs=ptA[:, 0:512], start=True, stop=False)
                        nc.tensor.matmul(
                            out=po[:, 128:512], lhsT=v_ap(h, 1),
                            rhs=ptA[:, 512:896], start=False, stop=False)
                        nc.tensor.matmul(
                            out=po[:, 256:512], lhsT=v_ap(h, 2),
                            rhs=ptB[:, 0:256], start=False, stop=False)
                        nc.tensor.matmul(
                            out=po[:, 384:512], lhsT=v_ap(h, 3),
                            rhs=ptB[:, 256:384], start=False, stop=True)
                    else:
                        po = po_t[h]
                        nc.tensor.matmul(
                            out=po[:, 0:128], lhsT=v_ap(h, 4 * c),
                            rhs=ptA[:, 0:128], start=False, stop=False)
                        nc.tensor.matmul(
                            out=po[:, 128:256], lhsT=v_ap(h, 4 * c + 1),
                            rhs=ptA[:, 512:640], start=False, stop=False)
                        nc.tensor.matmul(
                            out=po[:, 256:384], lhsT=v_ap(h, 4 * c + 2),
                            rhs=ptB[:, 0:128], start=False, stop=False)
                        nc.tensor.matmul(
                            out=po[:, 384:512], lhsT=v_ap(h, 4 * c + 3),
                            rhs=ptB[:, 256:384], start=False, stop=True)

                def stage_posb(h):
                    """po -> SBUF; frees the PSUM bank fast and gives the
                    reciprocal an SBUF source (PSUM input diverges on HW)."""
                    po = po_t[h]
                    posb = posp.tile([65, 512], F32, tag="pos", name=f"ps{c}{h}")
                    if c == 3:  # ACT is exp-free by the chunk-3 tails
                        nc.scalar.activation(out=posb[:], in_=po[:, :], func=COPY)
                    else:
                        nc.vector.tensor_copy(out=posb[:], in_=po[:, :])
                    return posb

                def make_norm(h):
                    def norm():
                        po = po_t[h]
                        # den must land in a partition-0 SBUF tile via plain
                        # tensor_copy: the custom-DVE reciprocal misreads
                        # PSUM and partition-offset inputs on real HW
                        den = recp.tile([1, 512], F32, tag="den", name=f"dn{c}{h}")
                        nc.vector.tensor_copy(out=den[:], in_=po[64:65, :])
                        posb = stage_posb(h)
                        rec = recp.tile([1, 512], F32, tag="rec", name=f"rc{c}{h}")
                        nc.vector.reciprocal_approx_fast(out=rec[:], in_=den[:])
                        recb = recp.tile([1, 512], BF16, tag="recb",
                                         name=f"rb{c}{h}")
                        if c == 3:
                            nc.scalar.activation(out=recb[:], in_=rec[:],
                                                 func=COPY)
                        else:
                            nc.vector.tensor_copy(out=recb[:], in_=rec[:])
                        # broadcast via PE rank-1 outer product: ones65^T @ recb
                        bc = pop.tile([65, 512], F32, tag="po", name=f"bc{c}{h}")
                        nc.tensor.matmul(out=bc[:, :], lhsT=ones65[:],
                                         rhs=recb[:], start=True, stop=True)
                        if c == 3:
                            for t4 in range(4):
                                cs = slice(128 * t4, 128 * (t4 + 1))
                                if h == 0:
                                    nc.vector.tensor_mul(
                                        out=onp3t[t4][0:64, :],
                                        in0=posb[0:64, cs], in1=bc[0:64, cs])
                                elif h == 1:
                                    nc.vector.tensor_mul(
                                        out=onp3t[t4][64:128, :],
                                        in0=posb[0:64, cs], in1=bc[0:64, cs])
                                else:
                                    nc.vector.tensor_mul(
                                        out=ons3t[t4][:],
                                        in0=posb[:, cs], in1=bc[:, cs])
                        elif h == 0:
                            nc.vector.tensor_mul(out=onp[c][0:64, :],
                                                 in0=posb[0:64, :],
                                                 in1=bc[0:64, :])
                        elif h == 1:
                            nc.vector.tensor_mul(out=onp[c][64:128, :],
                                                 in0=posb[0:64, :],
                                                 in1=bc[0:64, :])
                        else:
                            nc.vector.tensor_mul(out=ons[c][:],
                                                 in0=posb[:, :], in1=bc[:, :])
                    return norm

                def make_norm_pieces(h):
                    """Last norm of the kernel, split into per-128-column
                    pieces so op12..15 start as each piece lands.  All po
                    reads (den pieces + posb) are emitted HERE, before any
                    later pop-pool alloc can reuse po's PSUM buffer."""
                    po = po_t[h]
                    dens = []
                    for t4 in range(4):
                        den = recp.tile([1, 128], F32, tag=f"den4_{t4}",
                                        name=f"dn4_{t4}")
                        nc.vector.tensor_copy(
                            out=den[:], in_=po[64:65, 128 * t4:128 * (t4 + 1)])
                        dens.append(den)
                    posb = stage_posb(h)

                    def piece(t4):
                        cs = slice(128 * t4, 128 * (t4 + 1))
                        rec = recp.tile([1, 128], F32, tag="rec4",
                                        name=f"rc4_{t4}")
                        nc.vector.reciprocal_approx_fast(out=rec[:],
                                                         in_=dens[t4][:])
                        recb = recp.tile([1, 128], BF16, tag="recb4",
                                         name=f"rb4_{t4}")
                        nc.scalar.activation(out=recb[:], in_=rec[:], func=COPY)
                        bc = pop.tile([65, 512], F32, tag="po", name=f"bc4_{t4}")
                        nc.tensor.matmul(out=bc[:, 0:128], lhsT=ones65[:],
                                         rhs=recb[:], start=True, stop=True)
                        nc.vector.tensor_mul(out=ons3t[t4][:],
                                             in0=posb[:, cs], in1=bc[:, 0:128])
                    return [lambda t4=t4: piece(t4) for t4 in range(4)]

                def emit_tail(h):
                    tail_pv(h)
                    make_norm(h)()

                units = [(h, p) for h in range(HPC) for p in range(npairs)]
                nu = len(units)
                nf = len(fillers)
                fill_at = {}
                for k in range(nf):
                    fill_at.setdefault(
                        min(nu - 1, (k + 1) * nu // (nf + 1)), []).append(k)
                pend = deque()
                tails = deque()

                def pop_one():
                    h, p = pend.popleft()
                    emit_P_main(h, p)
                    if p == npairs - 1:
                        tails.append(h)
                    elif p == 1 and tails:
                        emit_tail(tails.popleft())

                for i, u in enumerate(units):
                    emit_S(*u)
                    pend.append(u)
                    if len(pend) > LAG:
                        pop_one()
                    if i == 1 and pre is not None:
                        pre()
                    for k in fill_at.get(i, ()):
                        fillers[k]()
                while pend:
                    pop_one()
                # drain: remaining tails, with fillers covering the last
                # tail's slow normalize chain
                dfill = list(drain_fillers)
                hs = list(tails)
                norms = []
                for idx, h in enumerate(hs):
                    tail_pv(h)
                    if idx == len(hs) - 1 and last_pieces:
                        if idx >= 1:
                            norms[idx - 1]()
                        pieces = make_norm_pieces(h)   # emits posb now
                        for f in dfill[idx:]:
                            f()
                        return pieces
                    norms.append(make_norm(h))
                    if idx >= 1:
                        norms[idx - 1]()
                    if idx < len(dfill):
                        dfill[idx]()
                for f in dfill[len(hs):]:
                    f()
                return norms[-1]

            # proj(1..3) ride as fillers one phase ahead of their att —
            # the head DMA wall (~220GB/s) can't feed a serial proj(1)
            warm(4)
            proj_kmajor(0)
            nrm = att(0, proj_groups(1)[0:5],
                      drain_fillers=proj_groups(1)[5:7])
            nrm = att(1, proj_groups(2)[0:6], pre=nrm,
                      drain_fillers=proj_groups(2)[6:7])
            nrm = att(2, proj_groups(3)
                      + [lambda t=t: outproj(t) for t in (0, 1)], pre=nrm,
                      drain_fillers=[lambda: outproj(2), lambda: outproj(3)])
            pieces = att(3, [lambda t=t: outproj(t) for t in (4, 5, 6, 7, 8, 9)],
                         pre=nrm,
                         drain_fillers=[lambda: outproj(10),
                                        lambda: outproj(11)],
                         last_pieces=True)
            for t, pc in zip((12, 13, 14, 15), pieces):
                pc()
                outproj(t, tail=True)

    nc.compile()
    return nc


def make_inputs(x, attention_mask, Wq, Wk, Wv, Wo, bo, use_pbias):
    bf = ml_dtypes.bfloat16
    kk = np.arange(128)[:, None]
    qq = np.arange(128)[None, :]
    tri01 = (qq >= kk).astype(np.float32)
    tri2 = np.repeat(tri01[:, None, :], 2, axis=1)

    def split_k(arr):  # [768, C] -> [128, 6, C]
        return np.ascontiguousarray(
            arr.reshape(KT, 128, arr.shape[1]).transpose(1, 0, 2))

    in_maps = []
    for core in range(NCORES):
        b, g = core // 4, core % 4
        h0, h1, h2 = range(HPC * g, HPC * (g + 1))
        xTb = split_k(np.ascontiguousarray(x[b].T)).astype(bf)
        wqk = np.empty((D, 384), np.float32)
        wqk[:, 0:64] = Wq[HD * h0:HD * (h0 + 1), :].T
        wqk[:, 64:128] = Wq[HD * h1:HD * (h1 + 1), :].T
        wqk[:, 128:192] = Wk[HD * h0:HD * (h0 + 1), :].T
        wqk[:, 192:256] = Wk[HD * h1:HD * (h1 + 1), :].T
        wqk[:, 256:320] = Wq[HD * h2:HD * (h2 + 1), :].T
        wqk[:, 320:384] = Wk[HD * h2:HD * (h2 + 1), :].T
        wv_ = Wv[HD * h0:HD * (h2 + 1), :].T
        wop = np.concatenate(
            [Wo[:, HD * h0:HD * (h0 + 1)].T, Wo[:, HD * h1:HD * (h1 + 1)].T])
        wos = np.zeros((65, D), np.float32)
        wos[0:64] = Wo[:, HD * h2:HD * (h2 + 1)].T
        if g == 0:
            wos[64] = bo
        m = {"wqk": split_k(wqk).astype(bf),
             "wv": split_k(np.ascontiguousarray(wv_)).astype(bf),
             "wop": wop.astype(bf),
             "wos": wos.astype(bf),
             "trid": tri2.astype(bf)}
        for c in range(CH):
            m[f"xT{c}"] = np.ascontiguousarray(xTb[:, :, 512 * c:512 * (c + 1)])
        if use_pbias:
            pb = ((1.0 - attention_mask[b].astype(np.float32)) * NEG)
            m["pbias"] = np.ascontiguousarray(pb.reshape(SQT, 128).T)
        in_maps.append(m)
    return in_maps


_NC_CACHE = {}


def _get_nc(use_pbias):
    key = ("nc", use_pbias)
    if key not in _NC_CACHE:
        _NC_CACHE[key] = build_nc(use_pbias)
    return _NC_CACHE[key]


def kernel(x, attention_mask, Wq, Wk, Wv, Wo, bo, _trace=False, _trace_kwargs=None):
    x = np.asarray(x, np.float32)
    attention_mask = np.asarray(attention_mask, np.float32)
    Wq, Wk, Wv, Wo, bo = (np.asarray(a, np.float32) for a in (Wq, Wk, Wv, Wo, bo))
    use_pbias = not bool(np.all(attention_mask == 1.0))
    nc = _get_nc(use_pbias)
    in_maps = make_inputs(x, attention_mask, Wq, Wk, Wv, Wo, bo, use_pbias)
    res = run_bass_kernel_spmd(nc, in_maps, list(range(NCORES)),
                               trace=_trace, **(_trace_kwargs or {}))
    parts = [np.asarray(res.results[i]["y"]).astype(np.float32)
             for i in range(NCORES)]
    out = np.stack([sum(parts[0:4]), sum(parts[4:8])])
    if _trace:
        return out, res
    return out


# revision 25
# speedup vs baseline: 1.0289x; 1.0289x over previous
"""GPT-Neo self-attention on 8 NeuronCores (Trainium2, Bass/Tile) — v11.

Sharding: core i handles batch i//4 and head-group i%4 (3 of 12 heads).
Each core computes a partial out-projection [S, D] (bf16); host sums the
4 partials per batch in f32.

v11 vs v10 (all driven by the HW trace):
- Head: inputs stream as per-k-tile DMA slices on FOUR queues
  (sync/vector/gpsimd/scalar); proj(0)/proj(1) accumulate k-major so the
  first matmul fires as soon as slice k=0 lands (~6us vs 13.2us).
  Warmup matmuls during the DMA wait keep the PE DVFS clock hot.
- PE p-state: every semaphore wait on the PE resets its DVFS ramp
  (observed quantized 216/318/427ns for N=512).  So: LAG 4->6, the two
  Pool tri-mask multiplies per diag pair merged into ONE strided-AP
  multiply, diag-B score sections packed contiguously (1 exp not 2),
  posb/copies moved off the hot engines, `den` staging copy dropped
  (reciprocal reads PSUM row directly).
- Phase boundaries: the last tail's normalize chain (~2us serial) is
  deferred into the next att() phase; att(3)'s drain is filled with
  outproj work.
- Endgame: chunk-3 onp/ons live as per-128-column tiles so op12..15
  start as soon as the first column piece is normalized.
"""

import numpy as np
import ml_dtypes
from collections import deque
from contextlib import ExitStack

import concourse.bass as bass
from concourse import bacc
import concourse.mybir as mybir
import concourse.tile as tile
from concourse.bass_utils import run_bass_kernel_spmd

B, S, D, H = 2, 2048, 768, 12
HD = 64
HPC = 3
NCORES = 8
NEG = -1.0e30
F32 = mybir.dt.float32
BF16 = mybir.dt.bfloat16
EXP = mybir.ActivationFunctionType.Exp
COPY = mybir.ActivationFunctionType.Copy

KT = D // 128
SQT = S // 128
CH = S // 512
LAG = 6          # main-PV pair-units behind scores
WARM = 8         # PE warmup matmuls during the head DMA wait


def build_nc(use_pbias=False):
    nc = bacc.Bacc(None, target_bir_lowering=False)

    xT = [nc.declare_dram_parameter(f"xT{c}", [128, KT, 512], BF16,
                                    isOutput=False) for c in range(CH)]
    wqk = nc.declare_dram_parameter("wqk", [128, KT, 384], BF16, isOutput=False)
    wv = nc.declare_dram_parameter("wv", [128, KT, 192], BF16, isOutput=False)
    wop = nc.declare_dram_parameter("wop", [128, D], BF16, isOutput=False)
    wos = nc.declare_dram_parameter("wos", [65, D], BF16, isOutput=False)
    trid = nc.declare_dram_parameter("trid", [128, 2, 128], BF16, isOutput=False)
    if use_pbias:
        pbias = nc.declare_dram_parameter("pbias", [128, SQT], F32, isOutput=False)
    y = nc.declare_dram_parameter("y", [S, D], BF16, isOutput=True)

    with tile.TileContext(nc) as tc:
        with ExitStack() as ctx:
            persist = ctx.enter_context(tc.tile_pool(name="persist", bufs=1))
            ptp = ctx.enter_context(tc.tile_pool(name="ptp", bufs=14))
            recp = ctx.enter_context(tc.tile_pool(name="recp", bufs=3))
            posp = ctx.enter_context(tc.tile_pool(name="posp", bufs=2))
            big = ctx.enter_context(tc.tile_pool(name="big", bufs=3, space="PSUM"))
            pop = ctx.enter_context(tc.tile_pool(name="pop", bufs=2, space="PSUM"))

            # per-(chunk,k) x tiles so k-major proj never waits a later slice
            xck = [[persist.tile([128, 512], BF16, tag=f"x{c}_{k}",
                                 name=f"x{c}_{k}") for k in range(KT)]
                   for c in range(2)]
            xc = [None, None] + [persist.tile([128, KT, 512], BF16, tag=f"xc{c}",
                                              name=f"xc{c}") for c in (2, 3)]
            wqk_k = [persist.tile([128, 384], BF16, tag=f"wqk{k}",
                                  name=f"wqk{k}") for k in range(KT)]
            wv_sb = persist.tile([128, KT, 192], BF16, tag="wv", name="wv")
            wop_sb = persist.tile([128, D], BF16, tag="wop", name="wop")
            wos_sb = persist.tile([65, D], BF16, tag="wos", name="wos")
            tri_sb = persist.tile([128, 2, 128], BF16, tag="tri", name="tri")
            ones65 = persist.tile([1, 65], BF16, tag="ones65", name="ones65")
            wsrc = persist.tile([128, 256], BF16, tag="wsrc", name="wsrc")
            if use_pbias:
                pb_sb = persist.tile([128, SQT], F32, tag="pb", name="pb")
            # per-chunk activation tiles (avoid false tile-level deps)
            q01c = [persist.tile([128, 512], BF16, tag=f"q01_{c}", name=f"q01_{c}")
                    for c in range(CH)]
            k01c = [persist.tile([128, 512], BF16, tag=f"k01_{c}", name=f"k01_{c}")
                    for c in range(CH)]
            q2c = [persist.tile([64, 512], BF16, tag=f"q2_{c}", name=f"q2_{c}")
                   for c in range(CH)]
            k2c = [persist.tile([64, 512], BF16, tag=f"k2_{c}", name=f"k2_{c}")
                   for c in range(CH)]
            vc = [persist.tile([128, HPC, 4, 65], BF16, tag=f"v{c}", name=f"v{c}")
                  for c in range(CH)]
            onp = [persist.tile([128, 512], BF16, tag=f"onp{c}", name=f"onp{c}")
                   for c in range(CH - 1)]
            ons = [persist.tile([65, 512], BF16, tag=f"ons{c}", name=f"ons{c}")
                   for c in range(CH - 1)]
            # chunk 3: per-128-col pieces so op12..15 start per piece
            onp3t = [persist.tile([128, 128], BF16, tag=f"onp3_{t}",
                                  name=f"onp3_{t}") for t in range(4)]
            ons3t = [persist.tile([65, 128], BF16, tag=f"ons3_{t}",
                                  name=f"ons3_{t}") for t in range(4)]
            otc = [persist.tile([128, 4, D], BF16, tag=f"ot{c}", name=f"ot{c}")
                   for c in range(CH)]

            # ---- head DMA plan ----
            # sync + scalar are HW-DGE (low latency) -> the critical per-k
            # slices; gpsimd is SW-DGE (~7us latency) -> latency-tolerant
            # bulk only.  DVE queue leads with the warmup-source memset.
            nc.vector.memset(wsrc[:], 0.25)
            nc.vector.memset(vc[0][:], 1.0)
            for k in range(KT):                       # proj(0) inputs
                (nc.sync if k % 2 == 0 else nc.scalar).dma_start(
                    out=wqk_k[k][:], in_=wqk[:, k, :])
                (nc.scalar if k % 2 == 0 else nc.sync).dma_start(
                    out=xck[0][k][:], in_=xT[0][:, k, :])
            for k in range(KT):                       # proj(1) inputs
                (nc.sync if k % 2 == 0 else nc.scalar).dma_start(
                    out=xck[1][k][:], in_=xT[1][:, k, :])
            # SW-DGE (gpsimd) runs ~20GB/s serial with ~7us latency: give it
            # only wv + late weights; everything x goes on the HW rings
            nc.gpsimd.dma_start(out=wv_sb[:], in_=wv[:, :, :])
            nc.sync.dma_start(out=tri_sb[:], in_=trid[:, :, :])
            nc.sync.dma_start(out=xc[2][:, 0:3, :], in_=xT[2][:, 0:3, :])
            nc.scalar.dma_start(out=xc[2][:, 3:6, :], in_=xT[2][:, 3:6, :])
            nc.sync.dma_start(out=xc[3][:, 0:3, :], in_=xT[3][:, 0:3, :])
            nc.scalar.dma_start(out=xc[3][:, 3:6, :], in_=xT[3][:, 3:6, :])
            nc.gpsimd.dma_start(out=wop_sb[:], in_=wop[:, :])
            nc.gpsimd.dma_start(out=wos_sb[:], in_=wos[:, :])
            if use_pbias:
                nc.scalar.dma_start(out=pb_sb[:], in_=pbias[:, :])
            for c in range(1, CH):
                nc.vector.memset(vc[c][:], 1.0)
            nc.vector.memset(ones65[:], 1.0)

            _warm_n = [0]

            def warm(n):
                """Junk matmuls that keep the PE DVFS ramp hot during DMA
                waits; WAW on the rotating pop bufs is in-order on PE."""
                for _ in range(n):
                    w = _warm_n[0]
                    _warm_n[0] += 1
                    wps = pop.tile([65, 512], F32, tag="po", name=f"warm{w}")
                    nc.tensor.matmul(out=wps[:, 0:256], lhsT=wsrc[:, 0:65],
                                     rhs=wsrc[:, :], start=True, stop=True)

            def proj_kmajor(c):
                """k-major qkv projection for chunk c, DMA-paced: warmup
                matmuls between k groups keep the clock up during waits."""
                ps = [big.tile([128, 1024], F32, tag="big", name=f"p{off}_{c}")
                      for off in (0, 128, 256)]
                for k in range(KT):
                    for gi, off in enumerate((0, 128, 256)):
                        nc.tensor.matmul(
                            out=ps[gi][:, 0:512],
                            lhsT=wqk_k[k][:, off:off + 128],
                            rhs=xck[c][k][:],
                            start=(k == 0), stop=(k == KT - 1))
                    if k < KT - 1:
                        warm(4 if k < 4 else 2)
                # q01 gates the pv psum-buf reuse: DVE's queue is free
                # earliest at the head (gpsimd cannot read PSUM)
                nc.vector.tensor_copy(out=q01c[c][:], in_=ps[0][:, 0:512])
                nc.scalar.copy(out=k01c[c][:], in_=ps[1][:, 0:512])
                nc.scalar.copy(out=q2c[c][:], in_=ps[2][0:64, 0:512])
                nc.vector.tensor_copy(out=k2c[c][:], in_=ps[2][64:128, 0:512])
                warm(2)  # cover the wv-DMA / q01-copy wait before v groups
                for jj in range(4):
                    pv = big.tile([128, 1024], F32, tag="big", name=f"pv{c}_{jj}")
                    for k in range(KT):
                        nc.tensor.matmul(
                            out=pv[:, 0:192],
                            lhsT=xck[c][k][:, 128 * jj:128 * (jj + 1)],
                            rhs=wv_sb[:, k, :],
                            start=(k == 0), stop=(k == KT - 1))
                    for h in range(HPC):
                        nc.vector.tensor_copy(out=vc[c][:, h, jj, 0:64],
                                              in_=pv[:, 64 * h:64 * (h + 1)])

            def xap(c, k, lo=0, hi=512):
                return (xck[c][k][:, lo:hi] if c < 2
                        else xc[c][:, k, lo:hi])

            def proj_qk_group(c, off):
                ps = big.tile([128, 1024], F32, tag="big", name=f"p{off}_{c}")
                for k in range(KT):
                    nc.tensor.matmul(
                        out=ps[:, 0:512],
                        lhsT=wqk_k[k][:, off:off + 128],
                        rhs=xap(c, k),
                        start=(k == 0), stop=(k == KT - 1))
                if off == 0:
                    nc.vector.tensor_copy(out=q01c[c][:], in_=ps[:, 0:512])
                elif off == 128:
                    nc.vector.tensor_copy(out=k01c[c][:], in_=ps[:, 0:512])
                else:
                    nc.vector.tensor_copy(out=q2c[c][:], in_=ps[0:64, 0:512])
                    nc.vector.tensor_copy(out=k2c[c][:], in_=ps[64:128, 0:512])

            def proj_v_group(c, jj):
                pv = big.tile([128, 1024], F32, tag="big", name=f"pv{c}_{jj}")
                for k in range(KT):
                    nc.tensor.matmul(
                        out=pv[:, 0:192],
                        lhsT=xap(c, k, 128 * jj, 128 * (jj + 1)),
                        rhs=wv_sb[:, k, :],
                        start=(k == 0), stop=(k == KT - 1))
                for h in range(HPC):
                    nc.vector.tensor_copy(out=vc[c][:, h, jj, 0:64],
                                          in_=pv[:, 64 * h:64 * (h + 1)])

            def proj_groups(c):
                gs = [lambda off=off: proj_qk_group(c, off)
                      for off in (0, 128, 256)]
                gs += [lambda jj=jj: proj_v_group(c, jj) for jj in range(4)]
                return gs

            def kq(h, j):
                """(k-block lhsT) for head h, key tile j."""
                cj, jj = j // 4, j % 4
                if h == 2:
                    return k2c[cj][:, 128 * jj:128 * (jj + 1)]
                lo = 64 * h
                return k01c[cj][lo:lo + 64, 128 * jj:128 * (jj + 1)]

            def qv(h, c, lo, hi):
                if h == 2:
                    return q2c[c][:, lo:hi]
                p0 = 64 * h
                return q01c[c][p0:p0 + 64, lo:hi]

            def v_ap(h, j):
                return vc[j // 4][:, h, j % 4, :]

            def exp_emit(pt, Sg, sections):
                if use_pbias:
                    for lo, hi, j in sections:
                        nc.scalar.activation(out=pt[:, lo:hi], in_=Sg[:, lo:hi],
                                             func=EXP, bias=pb_sb[:, j:j + 1])
                else:
                    lo, hi = sections[0][0], sections[-1][1]
                    nc.scalar.activation(out=pt[:, lo:hi], in_=Sg[:, lo:hi],
                                         func=EXP)

            def tri_mask(pt, stride):
                """one Pool multiply masking cols {0:128, stride:stride+128}"""
                v = pt[:].rearrange("p (a b) -> p a b", b=stride)[:, 0:2, 0:128]
                nc.gpsimd.tensor_mul(out=v, in0=v, in1=tri_sb[:])

            def outproj(t, tail=False):
                c_, tt = t // 4, t % 4
                Sg = big.tile([128, 1024], F32, tag="big", name=f"op{t}")
                if c_ == 3:
                    lp, ls = onp3t[tt][:], ons3t[tt][:]
                else:
                    ts_ = slice(128 * tt, 128 * (tt + 1))
                    lp, ls = onp[c_][:, ts_], ons[c_][:, ts_]
                for lo in (0, 512):
                    hs = slice(384 * (lo // 512), 384 * (lo // 512) + 384)
                    nc.tensor.matmul(out=Sg[:, lo:lo + 384],
                                     lhsT=lp, rhs=wop_sb[:, hs],
                                     start=True, stop=False)
                    nc.tensor.matmul(out=Sg[:, lo:lo + 384],
                                     lhsT=ls, rhs=wos_sb[:, hs],
                                     start=False, stop=True)
                if tail:
                    # ACT is exp-free here: split the copies across engines
                    # and flush each q-tile as soon as it is staged
                    nc.scalar.activation(out=otc[c_][:, tt, 0:384],
                                         in_=Sg[:, 0:384], func=COPY)
                    nc.vector.tensor_copy(out=otc[c_][:, tt, 384:768],
                                          in_=Sg[:, 512:896])
                    nc.sync.dma_start(
                        out=y[128 * t:128 * (t + 1), :].rearrange(
                            "(t p) d -> p t d", p=128),
                        in_=otc[c_][:, tt:tt + 1, :])
                    return
                nc.vector.tensor_copy(out=otc[c_][:, tt, 0:384], in_=Sg[:, 0:384])
                nc.vector.tensor_copy(out=otc[c_][:, tt, 384:768],
                                      in_=Sg[:, 512:896])
                if tt % 2 == 1:  # flush 2 q-tiles
                    nc.sync.dma_start(
                        out=y[128 * (t - 1):128 * (t + 1), :].rearrange(
                            "(t p) d -> p t d", p=128),
                        in_=otc[c_][:, tt - 1:tt + 1, :])

            def att(c, fillers, pre=None, drain_fillers=(), last_pieces=False):
                npairs = 2 * c + 2
                pts = {}
                po_t = {}

                def emit_S(h, p):
                    Sg = big.tile([128, 1024], F32, tag="big", name=f"S{c}{h}{p}")
                    pt = ptp.tile([128, 1024], BF16, tag="pt", name=f"pt{c}{h}{p}")
                    if p < 2 * c:          # full pair: j = 2p, 2p+1
                        j0 = 2 * p
                        nc.tensor.matmul(
                            out=Sg[:, 0:512], lhsT=kq(h, j0),
                            rhs=qv(h, c, 0, 512), start=True, stop=True)
                        nc.tensor.matmul(
                            out=Sg[:, 512:1024], lhsT=kq(h, j0 + 1),
                            rhs=qv(h, c, 0, 512), start=True, stop=True)
                        exp_emit(pt, Sg, [(0, 512, j0), (512, 1024, j0 + 1)])
                    elif p == 2 * c:       # diag A: j=4c (512 cols), 4c+1 (384)
                        j0 = 4 * c
                        nc.tensor.matmul(
                            out=Sg[:, 0:512], lhsT=kq(h, j0),
                            rhs=qv(h, c, 0, 512), start=True, stop=True)
                        nc.tensor.matmul(
                            out=Sg[:, 512:896], lhsT=kq(h, j0 + 1),
                            rhs=qv(h, c, 128, 512), start=True, stop=True)
                        exp_emit(pt, Sg, [(0, 512, j0), (512, 896, j0 + 1)])
                        tri_mask(pt, 512)
                    else:                  # diag B: j=4c+2 (256 cols), 4c+3 (128)
                        j0 = 4 * c + 2
                        nc.tensor.matmul(
                            out=Sg[:, 0:256], lhsT=kq(h, j0),
                            rhs=qv(h, c, 256, 512), start=True, stop=True)
                        nc.tensor.matmul(
                            out=Sg[:, 256:384], lhsT=kq(h, j0 + 1),
                            rhs=qv(h, c, 384, 512), start=True, stop=True)
                        exp_emit(pt, Sg, [(0, 256, j0), (256, 384, j0 + 1)])
                        tri_mask(pt, 256)
                    pts[(h, p)] = pt

                def emit_P_main(h, p):
                    if c == 0:
                        return  # all PVs deferred (need the masked pt anyway)
                    pt = pts[(h, p)]
                    if p == 0:
                        po_t[h] = pop.tile([65, 512], F32, tag="po",
                                           name=f"po{c}_{h}")
                    po = po_t[h]
                    if p < 2 * c:
                        nc.tensor.matmul(
                            out=po[:, :], lhsT=v_ap(h, 2 * p),
                            rhs=pt[:, 0:512], start=(p == 0), stop=False)
                        nc.tensor.matmul(
                            out=po[:, :], lhsT=v_ap(h, 2 * p + 1),
                            rhs=pt[:, 512:1024], start=False, stop=False)
                        pts.pop((h, p))
                    elif p == 2 * c:
                        nc.tensor.matmul(
                            out=po[:, 128:512], lhsT=v_ap(h, 4 * c),
                            rhs=pt[:, 128:512], start=False, stop=False)
                        nc.tensor.matmul(
                            out=po[:, 256:512], lhsT=v_ap(h, 4 * c + 1),
                            rhs=pt[:, 640:896], start=False, stop=False)
                    else:
                        nc.tensor.matmul(
                            out=po[:, 384:512], lhsT=v_ap(h, 4 * c + 2),
                            rhs=pt[:, 128:256], start=False, stop=False)

                def tail_pv(h):
                    """Deferred triangle PVs (+ all PVs at c==0)."""
                    ptA = pts.pop((h, 2 * c))
                    ptB = pts.pop((h, 2 * c + 1))
                    if c == 0:
                        po_t[h] = pop.tile([65, 512], F32, tag="po",
                                           name=f"po{c}_{h}")
                        po = po_t[h]
                        nc.tensor.matmul(
                            out=po[:, 0:512], lhsT=v_ap(h, 0),
                            rhs=ptA[:, 0:512], start=True, stop=False)
                        nc.tensor.matmul(
                            out=po[:, 128:512], lhsT=v_ap(h, 1),
                            rhs=ptA[:, 512:896], start=False, stop=False)
                        nc.tensor.matmul(
                            out=po[:, 256:512], lhsT=v_ap(h, 2),
                            rhs=ptB[:, 0:256], start=False, stop=False)
                        nc.tensor.matmul(
                            out=po[:, 384:512], lhsT=v_ap(h, 3),
                            rhs=ptB[:, 256:384], start=False, stop=True)
                    else:
                        po = po_t[h]
                        nc.tensor.matmul(
                            out=po[:, 0:128], lhsT=v_ap(h, 4 * c),
                            rhs=ptA[:, 0:128], start=False, stop=False)
                        nc.tensor.matmul(
                            out=po[:, 128:256], lhsT=v_ap(h, 4 * c + 1),
                            rhs=ptA[:, 512:640], start=False, stop=False)
                        nc.tensor.matmul(
                            out=po[:, 256:384], lhsT=v_ap(h, 4 * c + 2),
                            rhs=ptB[:, 0:128], start=False, stop=False)
                        nc.tensor.matmul(
                            out=po[:, 384:512], lhsT=v_ap(h, 4 * c + 3),
                            rhs=ptB[:, 256:384], start=False, stop=True)

                def stage_posb(h):
                    """po -> SBUF; frees the PSUM bank fast and gives the
                    reciprocal an SBUF source (PSUM input diverges on HW)."""
                    po = po_t[h]
                    posb = posp.tile([65, 512], F32, tag="pos", name=f"ps{c}{h}")
                    if c == 3:  # ACT is exp-free by the chunk-3 tails
                        nc.scalar.activation(out=posb[:], in_=po[:, :], func=COPY)
                    else:
                        nc.vector.tensor_copy(out=posb[:], in_=po[:, :])
                    return posb

                def make_norm(h):
                    def norm():
                        po = po_t[h]
                        # den must land in a partition-0 SBUF tile via plain
                        # tensor_copy: the custom-DVE reciprocal misreads
                        # PSUM and partition-offset inputs on real HW
                        den = recp.tile([1, 512], F32, tag="den", name=f"dn{c}{h}")
                        nc.vector.tensor_copy(out=den[:], in_=po[64:65, :])
                        posb = stage_posb(h)
                        rec = recp.tile([1, 512], F32, tag="rec", name=f"rc{c}{h}")
                        nc.vector.reciprocal_approx_fast(out=rec[:], in_=den[:])
                        recb = recp.tile([1, 512], BF16, tag="recb",
                                         name=f"rb{c}{h}")
                        if c == 3:
                            nc.scalar.activation(out=recb[:], in_=rec[:],
                                                 func=COPY)
                        else:
                            nc.vector.tensor_copy(out=recb[:], in_=rec[:])
                        # broadcast via PE rank-1 outer product: ones65^T @ recb
                        bc = pop.tile([65, 512], F32, tag="po", name=f"bc{c}{h}")
                        nc.tensor.matmul(out=bc[:, :], lhsT=ones65[:],
                                         rhs=recb[:], start=True, stop=True)
                        if c == 3:
                            for t4 in range(4):
                                cs = slice(128 * t4, 128 * (t4 + 1))
                                if h == 0:
                                    nc.vector.tensor_mul(
                                        out=onp3t[t4][0:64, :],
                                        in0=posb[0:64, cs], in1=bc[0:64, cs])
                                elif h == 1:
                                    nc.vector.tensor_mul(
                                        out=onp3t[t4][64:128, :],
                                        in0=posb[0:64, cs], in1=bc[0:64, cs])
                                else:
                                    nc.vector.tensor_mul(
                                        out=ons3t[t4][:],
                                        in0=posb[:, cs], in1=bc[:, cs])
                        elif h == 0:
                            nc.vector.tensor_mul(out=onp[c][0:64, :],
                                                 in0=posb[0:64, :],
                                                 in1=bc[0:64, :])
                        elif h == 1:
                            nc.vector.tensor_mul(out=onp[c][64:128, :],
                                                 in0=posb[0:64, :],
                                                 in1=bc[0:64, :])
                        else:
                            nc.vector.tensor_mul(out=ons[c][:],
                                                 in0=posb[:, :], in1=bc[:, :])
                    return norm

                def make_norm_pieces(h):
                    """Last norm of the kernel, split into per-128-column
                    pieces so op12..15 start as each piece lands.  All po
                    reads (den pieces + posb) are emitted HERE, before any
                    later pop-pool alloc can reuse po's PSUM buffer."""
                    po = po_t[h]
                    dens = []
                    for t4 in range(4):
                        den = recp.tile([1, 128], F32, tag=f"den4_{t4}",
                                        name=f"dn4_{t4}")
                        nc.vector.tensor_copy(
                            out=den[:], in_=po[64:65, 128 * t4:128 * (t4 + 1)])
                        dens.append(den)
                    posb = stage_posb(h)

                    def piece(t4):
                        cs = slice(128 * t4, 128 * (t4 + 1))
                        rec = recp.tile([1, 128], F32, tag="rec4",
                                        name=f"rc4_{t4}")
                        nc.vector.reciprocal_approx_fast(out=rec[:],
                                                         in_=dens[t4][:])
                        recb = recp.tile([1, 128], BF16, tag="recb4",
                                         name=f"rb4_{t4}")
                        # DVE, not ACT: the ACT queue is still draining the
                        # last exps + posb when the pieces fire
                        nc.vector.tensor_copy(out=recb[:], in_=rec[:])
                        bc = pop.tile([65, 512], F32, tag="po", name=f"bc4_{t4}")
                        nc.tensor.matmul(out=bc[:, 0:128], lhsT=ones65[:],
                                         rhs=recb[:], start=True, stop=True)
                        nc.vector.tensor_mul(out=ons3t[t4][:],
                                             in0=posb[:, cs], in1=bc[:, 0:128])
                    return [lambda t4=t4: piece(t4) for t4 in range(4)]

                def emit_tail(h):
                    tail_pv(h)
                    make_norm(h)()

                units = [(h, p) for h in range(HPC) for p in range(npairs)]
                nu = len(units)
                nf = len(fillers)
                fill_at = {}
                for k in range(nf):
                    fill_at.setdefault(
                        min(nu - 1, (k + 1) * nu // (nf + 1)), []).append(k)
                pend = deque()
                tails = deque()

                def pop_one():
                    h, p = pend.popleft()
                    emit_P_main(h, p)
                    if p == npairs - 1:
                        tails.append(h)
                    elif p == 1 and tails:
                        emit_tail(tails.popleft())

                for i, u in enumerate(units):
                    emit_S(*u)
                    pend.append(u)
                    if len(pend) > LAG:
                        pop_one()
                    if i == 1 and pre is not None:
                        pre()
                    for k in fill_at.get(i, ()):
                        fillers[k]()
                while pend:
                    pop_one()
                # drain: remaining tails, with fillers covering the last
                # tail's slow normalize chain
                dfill = list(drain_fillers)
                hs = list(tails)
                norms = []
                for idx, h in enumerate(hs):
                    tail_pv(h)
                    if idx == len(hs) - 1 and last_pieces:
                        if idx >= 1:
                            norms[idx - 1]()
                        pieces = make_norm_pieces(h)   # emits posb now
                        for f in dfill[idx:]:
                            f()
                        return pieces
                    norms.append(make_norm(h))
                    if idx >= 1:
                        norms[idx - 1]()
                    if idx < len(dfill):
                        dfill[idx]()
                for f in dfill[len(hs):]:
                    f()
                return norms[-1]

            # proj(1..3) ride as fillers one phase ahead of their att —
            # the head DMA wall (~220GB/s) can't feed a serial proj(1)
            warm(4)
            proj_kmajor(0)
            nrm = att(0, proj_groups(1)[0:5],
                      drain_fillers=proj_groups(1)[5:7])
            nrm = att(1, proj_groups(2)[0:6], pre=nrm,
                      drain_fillers=proj_groups(2)[6:7])
            nrm = att(2, proj_groups(3)
                      + [lambda t=t: outproj(t) for t in (0, 1)], pre=nrm,
                      drain_fillers=[lambda: outproj(2), lambda: outproj(3)])
            pieces = att(3, [lambda t=t: outproj(t) for t in (4, 5, 6, 7, 8, 9)],
                         pre=nrm,
                         drain_fillers=[lambda: outproj(10),
                                        lambda: outproj(11)],
                         last_pieces=True)
            for t, pc in zip((12, 13, 14, 15), pieces):
                pc()
                outproj(t, tail=True)

    nc.compile()
    return nc


def make_inputs(x, attention_mask, Wq, Wk, Wv, Wo, bo, use_pbias):
    bf = ml_dtypes.bfloat16
    kk = np.arange(128)[:, None]
    qq = np.arange(128)[None, :]
    tri01 = (qq >= kk).astype(np.float32)
    tri2 = np.repeat(tri01[:, None, :], 2, axis=1)

    def split_k(arr):  # [768, C] -> [128, 6, C]
        return np.ascontiguousarray(
            arr.reshape(KT, 128, arr.shape[1]).transpose(1, 0, 2))

    in_maps = []
    for core in range(NCORES):
        b, g = core // 4, core % 4
        h0, h1, h2 = range(HPC * g, HPC * (g + 1))
        xTb = split_k(np.ascontiguousarray(x[b].T)).astype(bf)
        wqk = np.empty((D, 384), np.float32)
        wqk[:, 0:64] = Wq[HD * h0:HD * (h0 + 1), :].T
        wqk[:, 64:128] = Wq[HD * h1:HD * (h1 + 1), :].T
        wqk[:, 128:192] = Wk[HD * h0:HD * (h0 + 1), :].T
        wqk[:, 192:256] = Wk[HD * h1:HD * (h1 + 1), :].T
        wqk[:, 256:320] = Wq[HD * h2:HD * (h2 + 1), :].T
        wqk[:, 320:384] = Wk[HD * h2:HD * (h2 + 1), :].T
        wv_ = Wv[HD * h0:HD * (h2 + 1), :].T
        wop = np.concatenate(
            [Wo[:, HD * h0:HD * (h0 + 1)].T, Wo[:, HD * h1:HD * (h1 + 1)].T])
        wos = np.zeros((65, D), np.float32)
        wos[0:64] = Wo[:, HD * h2:HD * (h2 + 1)].T
        if g == 0:
            wos[64] = bo
        m = {"wqk": split_k(wqk).astype(bf),
             "wv": split_k(np.ascontiguousarray(wv_)).astype(bf),
             "wop": wop.astype(bf),
             "wos": wos.astype(bf),
             "trid": tri2.astype(bf)}
        for c in range(CH):
            m[f"xT{c}"] = np.ascontiguousarray(xTb[:, :, 512 * c:512 * (c + 1)])
        if use_pbias:
            pb = ((1.0 - attention_mask[b].astype(np.float32)) * NEG)
            m["pbias"] = np.ascontiguousarray(pb.reshape(SQT, 128).T)
        in_maps.append(m)
    return in_maps


_NC_CACHE = {}


def _get_nc(use_pbias):
    key = ("nc", use_pbias)
    if key not in _NC_CACHE:
        _NC_CACHE[key] = build_nc(use_pbias)
    return _NC_CACHE[key]


def kernel(x, attention_mask, Wq, Wk, Wv, Wo, bo, _trace=False, _trace_kwargs=None):
    x = np.asarray(x, np.float32)
    attention_mask = np.asarray(attention_mask, np.float32)
    Wq, Wk, Wv, Wo, bo = (np.asarray(a, np.float32) for a in (Wq, Wk, Wv, Wo, bo))
    use_pbias = not bool(np.all(attention_mask == 1.0))
    nc = _get_nc(use_pbias)
    in_maps = make_inputs(x, attention_mask, Wq, Wk, Wv, Wo, bo, use_pbias)
    res = run_bass_kernel_spmd(nc, in_maps, list(range(NCORES)),
                               trace=_trace, **(_trace_kwargs or {}))
    parts = [np.asarray(res.results[i]["y"]).astype(np.float32)
             for i in range(NCORES)]
    out = np.stack([sum(parts[0:4]), sum(parts[4:8])])
    if _trace:
        return out, res
    return out
